# revision 7
# baseline (speedup 1.0000x reference)
"""DistogramHead Trainium2 kernel.

Computes out[b, i, j] = relu(0.5*(s_i[b,i] + s_j[b,j]) + b_out) where
  s_i = (x @ w_i + b_i) @ w_out  = x @ v_i + c_i,   v_i = w_i @ w_out
  s_j = (x @ w_j + b_j) @ w_out  = x @ v_j + c_j    (exact linear fold)

Shapes: x (4, 4096, 256) f32 -> out (4, 4096, 4096) f32 (256 MB).
Memory-bound on the output write (32 MB per core at ~358 GB/s HBM).

Sharding over 8 cores: core c handles batch b = c//2, row half r = c%2,
producing the contiguous slab out[b, r*2048:(r+1)*2048, :] (32 MB/core).
Each core receives x[b] transposed (xallT) in 512-column chunks plus a
tiny per-core one-hot selection matrix SEL - one static SPMD program, no
per-core control flow.

Per-core pipeline:
  1. v_j, v_i columns via DVE multiply+reduce over w chunks (d on partitions).
  2. s rows via PE matmuls: lhsT = [v_j, v_i] (stationary, M=2), rhs = xT
     512-col chunks (moving), 2 d-chunk accumulation in PSUM. One pass gives
     s_j AND s_i for all 4096 tokens, in two halves for early output start.
  3. Rb (128, 4096) = s_j row broadcast to all partitions via
     gpsimd.partition_broadcast (SBUF->SBUF, no HBM traffic).
  4. bias cols: s_i row -> (32,128) SBUF rearrange DMA -> PE matmul with the
     per-core SEL matrix (transpose + own-row selection in one op) ->
     A = 0.5*s_i_own + (0.5*(c_i+c_j) + b_out).
  5. 32 half-tiles: ACT relu(0.5*Rb_half + A[:, t]) -> 1 MB DMA store.
"""

import numpy as np

B = 4
L = 4096
D = 256
H = 128
P = 128
NCORES = 8
ROWS_PER_CORE = L // 2          # 2048
NBLK_OWN = ROWS_PER_CORE // P   # 16
NBLK_ALL = L // P               # 32
HALF = L // 2                   # 2048
NCHUNK = 8                      # 512-col x chunks

_PROGRAM = None


def _build_program():
    import concourse.bacc as bacc
    import concourse.bass as bass
    import concourse.tile as tile
    from concourse import mybir

    f32 = mybir.dt.float32
    nc = bacc.Bacc(None)

    xc = nc.dram_tensor("xc", [P, NCHUNK, 2, 512], f32, kind="ExternalInput")
    wi = nc.dram_tensor("wi", [D, H], f32, kind="ExternalInput")
    wj = nc.dram_tensor("wj", [D, H], f32, kind="ExternalInput")
    bi = nc.dram_tensor("bi", [1, H], f32, kind="ExternalInput")
    bj = nc.dram_tensor("bj", [1, H], f32, kind="ExternalInput")
    wout = nc.dram_tensor("wout", [1, H], f32, kind="ExternalInput")
    bout = nc.dram_tensor("bout", [1, 1], f32, kind="ExternalInput")
    sel = nc.dram_tensor("sel", [NBLK_OWN, 2, NBLK_OWN], f32, kind="ExternalInput")
    out = nc.dram_tensor("out", [ROWS_PER_CORE, L], f32, kind="ExternalOutput")

    def dram_bcast(ap, nparts):
        """AP reading the same DRAM region once per partition (stride 0)."""
        return bass.AP(tensor=ap.tensor, offset=ap.offset, ap=[[0, nparts]] + list(ap.ap))

    with tile.TileContext(nc) as tc:
        with (
            tc.tile_pool(name="persist", bufs=1) as persist,
            tc.tile_pool(name="junkp", bufs=2) as junkp,
            tc.tile_pool(name="outp", bufs=4) as outp,
            tc.tile_pool(name="psum", bufs=2, space="PSUM") as psum,
        ):
            # ---- regular small loads first (HWDGE, complete fast) ----
            wi_sb = persist.tile([P, 2, H], f32)
            nc.sync.dma_start(out=wi_sb[:], in_=wi.rearrange("(c p) h -> p c h", p=P))
            wj_sb = persist.tile([P, 2, H], f32)
            nc.sync.dma_start(out=wj_sb[:], in_=wj.rearrange("(c p) h -> p c h", p=P))
            # ---- x loads: 8 chunks, 4 KB/partition contiguous descriptors ----
            xts = []
            for n in range(NCHUNK):
                xt = persist.tile([P, 2, 512], f32, tag=f"xt{n}")
                nc.sync.dma_start(out=xt[:], in_=xc[:, n, :, :])
                xts.append(xt)
            sel_sb = persist.tile([NBLK_OWN, 2, NBLK_OWN], f32)
            nc.sync.dma_start(out=sel_sb[:], in_=sel[:, :, :])

            # ---- tiny row loads (HWDGE) + gpsimd partition broadcasts ----
            wrow = persist.tile([1, 3 * H + 1], f32)
            nc.sync.dma_start(out=wrow[0:1, 0:H], in_=wout[0:1, :])
            nc.sync.dma_start(out=wrow[0:1, H : 2 * H], in_=bi[0:1, :])
            nc.sync.dma_start(out=wrow[0:1, 2 * H : 3 * H], in_=bj[0:1, :])
            nc.sync.dma_start(out=wrow[0:1, 3 * H : 3 * H + 1], in_=bout[0:1, :])
            wout_bc = persist.tile([P, H], f32)
            nc.gpsimd.partition_broadcast(wout_bc[:], wrow[0:1, 0:H])
            bi_bc = persist.tile([P, H], f32)
            nc.gpsimd.partition_broadcast(bi_bc[:], wrow[0:1, H : 2 * H])
            bj_bc = persist.tile([P, H], f32)
            nc.gpsimd.partition_broadcast(bj_bc[:], wrow[0:1, 2 * H : 3 * H])
            bout_col = persist.tile([P, 1], f32)
            nc.gpsimd.partition_broadcast(bout_col[:], wrow[0:1, 3 * H : 3 * H + 1])

            # ---- v columns: vcols[:, c, 0] = v_j chunk c, [:, c, 1] = v_i ----
            vcols = persist.tile([P, 2, 2], f32)
            for c in range(2):
                for slot, w_sb in ((0, wj_sb), (1, wi_sb)):
                    junk = junkp.tile([P, H], f32, tag="junk")
                    nc.vector.tensor_mul(junk[:], w_sb[:, c, :], wout_bc[:])
                    nc.vector.reduce_sum(vcols[:, c, slot : slot + 1], junk[:],
                                         axis=mybir.AxisListType.X)

            # c_i, c_j, const = 0.5*(c_i+c_j)+b_out (replicated per partition)
            ci_col = persist.tile([P, 1], f32)
            junk = junkp.tile([P, H], f32, tag="junk")
            nc.vector.tensor_mul(junk[:], bi_bc[:], wout_bc[:])
            nc.vector.reduce_sum(ci_col[:], junk[:], axis=mybir.AxisListType.X)
            cj_col = persist.tile([P, 1], f32)
            junk = junkp.tile([P, H], f32, tag="junk")
            nc.vector.tensor_mul(junk[:], bj_bc[:], wout_bc[:])
            nc.vector.reduce_sum(cj_col[:], junk[:], axis=mybir.AxisListType.X)
            const_col = persist.tile([P, 1], f32)
            nc.vector.tensor_add(const_col[:], ci_col[:], cj_col[:])
            nc.vector.tensor_scalar(
                out=const_col[:], in0=const_col[:],
                scalar1=0.5, scalar2=bout_col[:, 0:1],
                op0=mybir.AluOpType.mult, op1=mybir.AluOpType.add,
            )

            # ---- s rows via PE: lhsT = [v_j, v_i] (stationary), xT moving ----
            # rows_sb row 0 = s_j (all 4096), row 1 = s_i (all 4096)
            rows_sb = persist.tile([2, L], f32)
            rb = persist.tile([P, L], f32)

            for half in range(2):
                ps = psum.tile([2, HALF], f32, tag="ps")
                for n in range(4):
                    for c in range(2):
                        nc.tensor.matmul(
                            ps[:, n * 512 : (n + 1) * 512],
                            vcols[:, c, :],
                            xts[half * 4 + n][:, c, :],
                            start=(c == 0), stop=(c == 1),
                        )
                j0 = half * HALF
                nc.scalar.copy(rows_sb[0:2, j0 : j0 + HALF], ps[:])
                nc.gpsimd.partition_broadcast(
                    rb[:, j0 : j0 + HALF], rows_sb[0:1, j0 : j0 + HALF])

            # ---- bias cols: si halves -> (16,128) -> SEL matmuls -> A ----
            asel_ps = psum.tile([P, NBLK_OWN], f32, tag="ps")
            si16s = []
            for half in range(2):
                s16 = persist.tile([NBLK_OWN, P], f32, tag=f"si16_{half}")
                si16s.append(s16)
            for half in range(2):
                nc.sync.dma_start(
                    out=si16s[half][:],
                    in_=rows_sb[1:2, half * HALF : (half + 1) * HALF])
                nc.tensor.matmul(asel_ps[:], si16s[half][:], sel_sb[:, half, :],
                                 start=(half == 0), stop=(half == 1))
            a_cols = persist.tile([P, NBLK_OWN], f32)
            nc.vector.tensor_scalar(
                out=a_cols[:], in0=asel_ps[:],
                scalar1=0.5, scalar2=const_col[:, 0:1],
                op0=mybir.AluOpType.mult, op1=mybir.AluOpType.add,
            )

            # ---- output: 32 half tiles ----
            for half in range(2):
                j0 = half * HALF
                for t in range(NBLK_OWN):
                    ot = outp.tile([P, HALF], f32, tag="ot")
                    nc.scalar.activation(
                        ot[:], rb[:, j0 : j0 + HALF],
                        mybir.ActivationFunctionType.Relu,
                        bias=a_cols[:, t : t + 1], scale=0.5,
                    )
                    nc.sync.dma_start(
                        out=out[t * P : (t + 1) * P, j0 : j0 + HALF], in_=ot[:])

    nc.finalize()
    return nc


def _get_program():
    global _PROGRAM
    if _PROGRAM is None:
        _PROGRAM = _build_program()
    return _PROGRAM


def _run(inputs, trace=False):
    from concourse.bass_utils import run_bass_kernel_spmd

    x = np.asarray(inputs["x"], np.float32)
    w_i = np.ascontiguousarray(np.asarray(inputs["w_i"], np.float32))
    w_j = np.ascontiguousarray(np.asarray(inputs["w_j"], np.float32))
    b_i = np.asarray(inputs["b_i"], np.float32).reshape(1, H)
    b_j = np.asarray(inputs["b_j"], np.float32).reshape(1, H)
    w_out = np.asarray(inputs["w_out"], np.float32).reshape(1, H)
    b_out = np.asarray(inputs["b_out"], np.float32).reshape(1, 1)

    # pre-chunked xT: xcs[b][p, n, c, l] = x[b][n*512+l, c*128+p]
    xcs = [np.ascontiguousarray(
        x[b].T.reshape(2, P, NCHUNK, 512).transpose(1, 2, 0, 3)) for b in range(B)]
    eye = np.eye(NBLK_OWN, dtype=np.float32)
    sels = []
    for r in range(2):
        s = np.zeros((NBLK_OWN, 2, NBLK_OWN), np.float32)
        s[:, r, :] = eye
        sels.append(s)

    nc = _get_program()
    in_maps = []
    for c in range(NCORES):
        b, r = divmod(c, 2)
        in_maps.append({
            "xc": xcs[b], "sel": sels[r],
            "wi": w_i, "wj": w_j, "bi": b_i, "bj": b_j,
            "wout": w_out, "bout": b_out,
        })
    res = run_bass_kernel_spmd(nc, in_maps, core_ids=list(range(NCORES)), trace=trace)
    full = np.empty((B, L, L), np.float32)
    for c in range(NCORES):
        b, r = divmod(c, 2)
        full[b, r * ROWS_PER_CORE : (r + 1) * ROWS_PER_CORE, :] = res.results[c]["out"]
    return full, res


def kernel(**inputs):
    full, _ = _run(inputs, trace=False)
    return full


# revision 8
# speedup vs baseline: 1.0838x; 1.0838x over previous
"""DistogramHead Trainium2 kernel.

Computes out[b, i, j] = relu(0.5*(s_i[b,i] + s_j[b,j]) + b_out) where
  s_i = (x @ w_i + b_i) @ w_out  = x @ v_i + c_i,   v_i = w_i @ w_out
  s_j = (x @ w_j + b_j) @ w_out  = x @ v_j + c_j    (exact linear fold)

Shapes: x (4, 4096, 256) f32 -> out (4, 4096, 4096) f32 (256 MB).
Memory-bound on the output write (32 MB per core at ~358 GB/s HBM).

Sharding over 8 cores: core c handles batch b = c//2, row half r = c%2,
producing the contiguous slab out[b, r*2048:(r+1)*2048, :] (32 MB/core).
Each core receives x[b] transposed (xallT) in 512-column chunks plus a
tiny per-core one-hot selection matrix SEL - one static SPMD program, no
per-core control flow.

Per-core pipeline:
  1. v_j, v_i columns via DVE multiply+reduce over w chunks (d on partitions).
  2. s rows via PE matmuls: lhsT = [v_j, v_i] (stationary, M=2), rhs = xT
     512-col chunks (moving), 2 d-chunk accumulation in PSUM. One pass gives
     s_j AND s_i for all 4096 tokens, in two halves for early output start.
  3. Rb (128, 4096) = s_j row broadcast to all partitions via
     gpsimd.partition_broadcast (SBUF->SBUF, no HBM traffic).
  4. bias cols: s_i row -> (32,128) SBUF rearrange DMA -> PE matmul with the
     per-core SEL matrix (transpose + own-row selection in one op) ->
     A = 0.5*s_i_own + (0.5*(c_i+c_j) + b_out).
  5. 32 half-tiles: ACT relu(0.5*Rb_half + A[:, t]) -> 1 MB DMA store.
"""

import numpy as np

B = 4
L = 4096
D = 256
H = 128
P = 128
NCORES = 8
ROWS_PER_CORE = L // 2          # 2048
NBLK_OWN = ROWS_PER_CORE // P   # 16
NBLK_ALL = L // P               # 32
HALF = L // 2                   # 2048
NCHUNK = 8                      # 512-col x chunks

_PROGRAM = None


def _build_program():
    import concourse.bacc as bacc
    import concourse.bass as bass
    import concourse.tile as tile
    from concourse import mybir

    f32 = mybir.dt.float32
    nc = bacc.Bacc(None)

    xc = nc.dram_tensor("xc", [P, NCHUNK, 2, 512], f32, kind="ExternalInput")
    wi = nc.dram_tensor("wi", [D, H], f32, kind="ExternalInput")
    wj = nc.dram_tensor("wj", [D, H], f32, kind="ExternalInput")
    bi = nc.dram_tensor("bi", [1, H], f32, kind="ExternalInput")
    bj = nc.dram_tensor("bj", [1, H], f32, kind="ExternalInput")
    wout = nc.dram_tensor("wout", [1, H], f32, kind="ExternalInput")
    bout = nc.dram_tensor("bout", [1, 1], f32, kind="ExternalInput")
    sel = nc.dram_tensor("sel", [NBLK_OWN, 2, NBLK_OWN], f32, kind="ExternalInput")
    out = nc.dram_tensor("out", [ROWS_PER_CORE, L], f32, kind="ExternalOutput")

    def dram_bcast(ap, nparts):
        """AP reading the same DRAM region once per partition (stride 0)."""
        return bass.AP(tensor=ap.tensor, offset=ap.offset, ap=[[0, nparts]] + list(ap.ap))

    with tile.TileContext(nc) as tc:
        with (
            tc.tile_pool(name="persist", bufs=1) as persist,
            tc.tile_pool(name="junkp", bufs=2) as junkp,
            tc.tile_pool(name="outp", bufs=4) as outp,
            tc.tile_pool(name="psum", bufs=2, space="PSUM") as psum,
        ):
            # ---- regular small loads first (HWDGE, complete fast) ----
            wi_sb = persist.tile([P, 2, H], f32)
            nc.sync.dma_start(out=wi_sb[:], in_=wi.rearrange("(c p) h -> p c h", p=P))
            wj_sb = persist.tile([P, 2, H], f32)
            nc.sync.dma_start(out=wj_sb[:], in_=wj.rearrange("(c p) h -> p c h", p=P))
            # ---- tiny row loads (HWDGE) + gpsimd partition broadcasts ----
            wrow = persist.tile([1, 3 * H + 1], f32)
            nc.sync.dma_start(out=wrow[0:1, 0:H], in_=wout[0:1, :])
            nc.sync.dma_start(out=wrow[0:1, H : 2 * H], in_=bi[0:1, :])
            nc.sync.dma_start(out=wrow[0:1, 2 * H : 3 * H], in_=bj[0:1, :])
            nc.sync.dma_start(out=wrow[0:1, 3 * H : 3 * H + 1], in_=bout[0:1, :])
            wout_bc = persist.tile([P, H], f32)
            nc.gpsimd.partition_broadcast(wout_bc[:], wrow[0:1, 0:H])
            bi_bc = persist.tile([P, H], f32)
            nc.gpsimd.partition_broadcast(bi_bc[:], wrow[0:1, H : 2 * H])
            bj_bc = persist.tile([P, H], f32)
            nc.gpsimd.partition_broadcast(bj_bc[:], wrow[0:1, 2 * H : 3 * H])
            bout_col = persist.tile([P, 1], f32)
            nc.gpsimd.partition_broadcast(bout_col[:], wrow[0:1, 3 * H : 3 * H + 1])


            # ---- x loads: 8 chunks, 4 KB/partition contiguous descriptors ----
            xts = []
            for n in range(NCHUNK):
                xt = persist.tile([P, 2, 512], f32, tag=f"xt{n}")
                nc.sync.dma_start(out=xt[:], in_=xc[:, n, :, :])
                xts.append(xt)
            sel_sb = persist.tile([NBLK_OWN, 2, NBLK_OWN], f32)
            nc.sync.dma_start(out=sel_sb[:], in_=sel[:, :, :])

            # ---- v columns: vcols[:, c, 0] = v_j chunk c, [:, c, 1] = v_i ----
            vcols = persist.tile([P, 2, 2], f32)
            for c in range(2):
                for slot, w_sb in ((0, wj_sb), (1, wi_sb)):
                    junk = junkp.tile([P, H], f32, tag="junk")
                    nc.vector.tensor_mul(junk[:], w_sb[:, c, :], wout_bc[:])
                    nc.vector.reduce_sum(vcols[:, c, slot : slot + 1], junk[:],
                                         axis=mybir.AxisListType.X)

            # c_i, c_j, const = 0.5*(c_i+c_j)+b_out (replicated per partition)
            ci_col = persist.tile([P, 1], f32)
            junk = junkp.tile([P, H], f32, tag="junk")
            nc.vector.tensor_mul(junk[:], bi_bc[:], wout_bc[:])
            nc.vector.reduce_sum(ci_col[:], junk[:], axis=mybir.AxisListType.X)
            cj_col = persist.tile([P, 1], f32)
            junk = junkp.tile([P, H], f32, tag="junk")
            nc.vector.tensor_mul(junk[:], bj_bc[:], wout_bc[:])
            nc.vector.reduce_sum(cj_col[:], junk[:], axis=mybir.AxisListType.X)
            const_col = persist.tile([P, 1], f32)
            nc.vector.tensor_add(const_col[:], ci_col[:], cj_col[:])
            nc.vector.tensor_scalar(
                out=const_col[:], in0=const_col[:],
                scalar1=0.5, scalar2=bout_col[:, 0:1],
                op0=mybir.AluOpType.mult, op1=mybir.AluOpType.add,
            )

            # ---- s rows via PE: lhsT = [v_j, v_i] (stationary), xT moving ----
            # rows_sb row 0 = s_j (all 4096), row 1 = s_i (all 4096)
            rows_sb = persist.tile([2, L], f32)
            rb = persist.tile([P, L], f32)

            for half in range(2):
                ps = psum.tile([2, HALF], f32, tag="ps")
                for n in range(4):
                    for c in range(2):
                        nc.tensor.matmul(
                            ps[:, n * 512 : (n + 1) * 512],
                            vcols[:, c, :],
                            xts[half * 4 + n][:, c, :],
                            start=(c == 0), stop=(c == 1),
                        )
                j0 = half * HALF
                nc.scalar.copy(rows_sb[0:2, j0 : j0 + HALF], ps[:])
                nc.gpsimd.partition_broadcast(
                    rb[:, j0 : j0 + HALF], rows_sb[0:1, j0 : j0 + HALF])

            # ---- bias cols: si halves -> (16,128) -> SEL matmuls -> A ----
            asel_ps = psum.tile([P, NBLK_OWN], f32, tag="ps")
            si16s = []
            for half in range(2):
                s16 = persist.tile([NBLK_OWN, P], f32, tag=f"si16_{half}")
                si16s.append(s16)
            for half in range(2):
                nc.sync.dma_start(
                    out=si16s[half][:],
                    in_=rows_sb[1:2, half * HALF : (half + 1) * HALF])
                nc.tensor.matmul(asel_ps[:], si16s[half][:], sel_sb[:, half, :],
                                 start=(half == 0), stop=(half == 1))
            a_cols = persist.tile([P, NBLK_OWN], f32)
            nc.vector.tensor_scalar(
                out=a_cols[:], in0=asel_ps[:],
                scalar1=0.5, scalar2=const_col[:, 0:1],
                op0=mybir.AluOpType.mult, op1=mybir.AluOpType.add,
            )

            # ---- output: 32 half tiles ----
            for half in range(2):
                j0 = half * HALF
                for t in range(NBLK_OWN):
                    ot = outp.tile([P, HALF], f32, tag="ot")
                    nc.scalar.activation(
                        ot[:], rb[:, j0 : j0 + HALF],
                        mybir.ActivationFunctionType.Relu,
                        bias=a_cols[:, t : t + 1], scale=0.5,
                    )
                    nc.sync.dma_start(
                        out=out[t * P : (t + 1) * P, j0 : j0 + HALF], in_=ot[:])

    nc.finalize()
    return nc


def _get_program():
    global _PROGRAM
    if _PROGRAM is None:
        _PROGRAM = _build_program()
    return _PROGRAM


def _run(inputs, trace=False):
    from concourse.bass_utils import run_bass_kernel_spmd

    x = np.asarray(inputs["x"], np.float32)
    w_i = np.ascontiguousarray(np.asarray(inputs["w_i"], np.float32))
    w_j = np.ascontiguousarray(np.asarray(inputs["w_j"], np.float32))
    b_i = np.asarray(inputs["b_i"], np.float32).reshape(1, H)
    b_j = np.asarray(inputs["b_j"], np.float32).reshape(1, H)
    w_out = np.asarray(inputs["w_out"], np.float32).reshape(1, H)
    b_out = np.asarray(inputs["b_out"], np.float32).reshape(1, 1)

    # pre-chunked xT: xcs[b][p, n, c, l] = x[b][n*512+l, c*128+p]
    xcs = [np.ascontiguousarray(
        x[b].T.reshape(2, P, NCHUNK, 512).transpose(1, 2, 0, 3)) for b in range(B)]
    eye = np.eye(NBLK_OWN, dtype=np.float32)
    sels = []
    for r in range(2):
        s = np.zeros((NBLK_OWN, 2, NBLK_OWN), np.float32)
        s[:, r, :] = eye
        sels.append(s)

    nc = _get_program()
    in_maps = []
    for c in range(NCORES):
        b, r = divmod(c, 2)
        in_maps.append({
            "xc": xcs[b], "sel": sels[r],
            "wi": w_i, "wj": w_j, "bi": b_i, "bj": b_j,
            "wout": w_out, "bout": b_out,
        })
    res = run_bass_kernel_spmd(nc, in_maps, core_ids=list(range(NCORES)), trace=trace)
    full = np.empty((B, L, L), np.float32)
    for c in range(NCORES):
        b, r = divmod(c, 2)
        full[b, r * ROWS_PER_CORE : (r + 1) * ROWS_PER_CORE, :] = res.results[c]["out"]
    return full, res


def kernel(**inputs):
    full, _ = _run(inputs, trace=False)
    return full
